# revision 8
# baseline (speedup 1.0000x reference)
"""3-layer GCN (message passing) on 8 Trainium2 NeuronCores.

Strategy
--------
Nodes are split into 80 load-balanced bins (8 cores x 10 blocks, 125
nodes each, balanced by in-degree so every (core, block) has nearly the
same edge count). Per layer:

  1. sharded projection  p = h_local @ W  (PE, fp16), rows pre-scaled by
     dinv[node]  (GCN norm is separable: norm_e = dinv[src]*dinv[dst])
  2. AllGather p_scaled -> p_full [10000, D] fp16 in DRAM (every core)
  3. per dst-block: dma_gather of the block's edge source rows
     (512B rows), then PE matmuls against host-precomputed fp8 0/1
     one-hot scatter matrices (SBUF-resident, shared by all 3 layers),
     accumulating segment sums in PSUM
  4. epilogue: relu(psum * dinv[dst] + b)  (DVE/ACT)

Self-loops are folded in as ordinary edges (the separable scaling gives
exactly dinv^2). Layer 3 ends with a log-softmax over the 40 classes
(free-dim reductions) and each core stores its 1250x40 slice.
"""

import numpy as np
import ml_dtypes

N = 10000
E = 320000
D = 256
DOUT = 40
DPAD = 128  # layer-3 projected width (pad 40 -> 128 so gather rows are 256B)
NCORES = 8
NBLK = 10       # dst blocks per core
NPB = 125       # nodes per block
NBINS = NCORES * NBLK
GT = 8  # gather chunk size in 128-row tiles; HW limit is 1024 idxs/instruction

_CACHE = {}

# test.py sets TRACE=True (after installing the NTFF hook shim) to get a
# hardware profile; the graded path leaves this False.
TRACE = False
LAST_RESULT = None


def _build_program(nt):
    """Build the SPMD Bass program. `nt` = gather tiles (of 128 edge slots)
    per dst block — identical for every (core, block) by construction."""
    import concourse.bacc as bacc
    import concourse.mybir as mybir
    import concourse.tile as tile

    dt = mybir.dt
    ntt = NBLK * nt  # total tiles per core

    nc = bacc.Bacc("TRN2", target_bir_lowering=False, debug=False,
                   num_devices=NCORES)

    # ---- I/O ----
    xl_in = nc.dram_tensor("xl", [128, NBLK * D], dt.float32, kind="ExternalInput")
    w1_in = nc.dram_tensor("w1", [128, 2 * D], dt.float16, kind="ExternalInput")
    wh_in = nc.dram_tensor("wh", [128, 2 * D], dt.float16, kind="ExternalInput")
    w2_in = nc.dram_tensor("w2", [128, 2 * DPAD], dt.float16, kind="ExternalInput")
    b1_in = nc.dram_tensor("b1t", [128, D], dt.float32, kind="ExternalInput")
    bh_in = nc.dram_tensor("bht", [128, D], dt.float32, kind="ExternalInput")
    b2_in = nc.dram_tensor("b2t", [128, DOUT], dt.float32, kind="ExternalInput")
    dinv_in = nc.dram_tensor("dinvc", [128, NBLK], dt.float32, kind="ExternalInput")
    idx_in = nc.dram_tensor("idx", [128, ntt * 8], dt.int16, kind="ExternalInput")
    mt_in = nc.dram_tensor("mt", [128, ntt * 128], dt.float8e4, kind="ExternalInput")
    id_in = nc.dram_tensor("ident", [128, 128], dt.float16, kind="ExternalInput")
    out_t = nc.dram_tensor("out", [NBLK * NPB, DOUT], dt.float32,
                           kind="ExternalOutput")

    with tile.TileContext(nc) as tc:
        with (
            tc.tile_pool(name="const", bufs=1) as cpool,
            tc.tile_pool(name="hbuf", bufs=1) as hpool,
            tc.tile_pool(name="g", bufs=6) as gpool,
            tc.tile_pool(name="tp", bufs=4) as tpool,
            tc.tile_pool(name="psc", bufs=3) as ppool,
            tc.tile_pool(name="epi", bufs=3) as epool,
            tc.tile_pool(name="tp_ps", bufs=4, space="PSUM") as tp_ps,
            tc.tile_pool(name="proj_ps", bufs=2, space="PSUM") as proj_ps,
            tc.tile_pool(name="acc_ps", bufs=2, space="PSUM") as acc_ps,
            tc.tile_pool(name="dram", bufs=1, space="DRAM") as dram,
        ):
            # ---- load constants ----
            idx_sb = cpool.tile([128, ntt * 8], dt.int16)
            mt_sb = cpool.tile([128, ntt * 128], dt.float8e4)
            ident = cpool.tile([128, 128], dt.float16)
            w1_sb = cpool.tile([128, 2 * D], dt.float16)
            wh_sb = cpool.tile([128, 2 * D], dt.float16)
            w2_sb = cpool.tile([128, 2 * DPAD], dt.float16)
            b1_sb = cpool.tile([128, D], dt.float32)
            bh_sb = cpool.tile([128, D], dt.float32)
            b2_sb = cpool.tile([128, DOUT], dt.float32)
            dinv_sb = cpool.tile([128, NBLK], dt.float32)
            nc.sync.dma_start(idx_sb[:], idx_in[:])
            nc.sync.dma_start(mt_sb[:], mt_in[:])
            nc.sync.dma_start(ident[:], id_in[:])
            nc.sync.dma_start(w1_sb[:], w1_in[:])
            nc.sync.dma_start(wh_sb[:], wh_in[:])
            nc.sync.dma_start(w2_sb[:], w2_in[:])
            nc.sync.dma_start(b1_sb[:], b1_in[:])
            nc.sync.dma_start(bh_sb[:], bh_in[:])
            nc.sync.dma_start(b2_sb[:], b2_in[:])
            nc.sync.dma_start(dinv_sb[:], dinv_in[:])

            # ---- x load + fp16 convert ----
            x32 = hpool.tile([128, NBLK * D], dt.float32)
            nc.sync.dma_start(x32[:], xl_in[:])
            xh = hpool.tile([128, NBLK * D], dt.float16)
            nc.vector.tensor_copy(xh[:], x32[:])

            def project(src_sb, w_sb, dcols, ag_in, tag):
                """p_scaled[m] = (src[:, m] @ W) * dinv -> ag_in rows."""
                for m in range(NBLK):
                    p_ps = proj_ps.tile([128, dcols], mybir.dt.float32,
                                        name=f"p_ps_{tag}_{m}", tag="p_ps")
                    for kt in range(2):
                        t_ps = tp_ps.tile([128, 128], mybir.dt.float16,
                                          name=f"t_ps_{tag}_{m}_{kt}", tag="t_ps")
                        nc.tensor.transpose(
                            t_ps[:], src_sb[:, m * D + kt * 128: m * D + (kt + 1) * 128],
                            ident[:])
                        t_sb = tpool.tile([128, 128], mybir.dt.float16,
                                          name=f"t_sb_{tag}_{m}_{kt}", tag="t_sb")
                        nc.vector.tensor_copy(t_sb[:], t_ps[:])
                        nc.tensor.matmul(p_ps[:], t_sb[:],
                                         w_sb[:, kt * dcols:(kt + 1) * dcols],
                                         start=(kt == 0), stop=(kt == 1))
                    psc = ppool.tile([128, dcols], mybir.dt.float16,
                                     name=f"psc_{tag}_{m}", tag="psc")
                    nc.vector.tensor_scalar(psc[:], p_ps[:], dinv_sb[:, m:m + 1],
                                            None, mybir.AluOpType.mult)
                    nc.sync.dma_start(ag_in[m * NPB:(m + 1) * NPB, :], psc[:NPB, :])

            chunk_state = {}

            def gather_block(ag_out, b, dcols, tag):
                """One dst-block's segment-sum accumulation. Gathers are
                issued in GT-tile (<=1024 idx) chunks over the layer's
                global tile stream (HW limit: 1024 idxs per dma_gather)."""
                chunks = chunk_state.setdefault(tag, {})
                acc = acc_ps.tile([128, dcols], mybir.dt.float32,
                                  name=f"acc_{tag}_{b}", tag="acc")
                for t in range(nt):
                    tg = b * nt + t
                    c = tg // GT
                    if c not in chunks:
                        k = min(GT, ntt - c * GT)
                        g = gpool.tile([128, k, dcols], mybir.dt.float16,
                                       name=f"g_{tag}_{c}", tag="g")
                        nc.gpsimd.dma_gather(
                            g[:], ag_out[:],
                            idx_sb[:, c * GT * 8: c * GT * 8 + k * 8],
                            k * 128, k * 128, dcols)
                        chunks[c] = g
                    nc.tensor.matmul(acc[:], mt_sb[:, tg * 128:(tg + 1) * 128],
                                     chunks[c][:, tg % GT, :],
                                     start=(t == 0), stop=(t == nt - 1))
                return acc

            # ================= layer 1 =================
            ag1_in = dram.tile([NBLK * NPB, D], dt.float16)
            ag1_out = dram.tile([N, D], dt.float16)
            project(xh, w1_sb, D, ag1_in, "l1")
            nc.gpsimd.collective_compute(
                "AllGather", mybir.AluOpType.bypass,
                replica_groups=[list(range(NCORES))],
                ins=[ag1_in[:]], outs=[ag1_out[:]])

            h1 = hpool.tile([128, NBLK * D], dt.float16)
            for b in range(NBLK):
                acc = gather_block(ag1_out, b, D, "l1")
                y = epool.tile([128, D], mybir.dt.float32, name=f"y1_{b}", tag="y")
                nc.vector.tensor_scalar(y[:], acc[:], dinv_sb[:, b:b + 1],
                                        None, mybir.AluOpType.mult)
                z = epool.tile([128, D], mybir.dt.float32, name=f"z1_{b}", tag="z")
                nc.vector.tensor_add(z[:], y[:], b1_sb[:])
                nc.scalar.activation(h1[:, b * D:(b + 1) * D], z[:],
                                     mybir.ActivationFunctionType.Relu)

            # ================= layer 2 =================
            ag2_in = dram.tile([NBLK * NPB, D], dt.float16)
            ag2_out = dram.tile([N, D], dt.float16)
            project(h1, wh_sb, D, ag2_in, "l2")
            nc.gpsimd.collective_compute(
                "AllGather", mybir.AluOpType.bypass,
                replica_groups=[list(range(NCORES))],
                ins=[ag2_in[:]], outs=[ag2_out[:]])

            h2 = hpool.tile([128, NBLK * D], dt.float16)
            for b in range(NBLK):
                acc = gather_block(ag2_out, b, D, "l2")
                y = epool.tile([128, D], mybir.dt.float32, name=f"y2_{b}", tag="y")
                nc.vector.tensor_scalar(y[:], acc[:], dinv_sb[:, b:b + 1],
                                        None, mybir.AluOpType.mult)
                z = epool.tile([128, D], mybir.dt.float32, name=f"z2_{b}", tag="z")
                nc.vector.tensor_add(z[:], y[:], bh_sb[:])
                nc.scalar.activation(h2[:, b * D:(b + 1) * D], z[:],
                                     mybir.ActivationFunctionType.Relu)

            # ================= layer 3 =================
            ag3_in = dram.tile([NBLK * NPB, DPAD], dt.float16)
            ag3_out = dram.tile([N, DPAD], dt.float16)
            # project h2 into DPAD-wide padded logits
            for m in range(NBLK):
                p_ps = proj_ps.tile([128, DPAD], mybir.dt.float32,
                                    name=f"p_ps_l3_{m}", tag="p_ps")
                for kt in range(2):
                    t_ps = tp_ps.tile([128, 128], mybir.dt.float16,
                                      name=f"t_ps_l3_{m}_{kt}", tag="t_ps")
                    nc.tensor.transpose(
                        t_ps[:], h2[:, m * D + kt * 128: m * D + (kt + 1) * 128],
                        ident[:])
                    t_sb = tpool.tile([128, 128], mybir.dt.float16,
                                      name=f"t_sb_l3_{m}_{kt}", tag="t_sb")
                    nc.vector.tensor_copy(t_sb[:], t_ps[:])
                    nc.tensor.matmul(p_ps[:], t_sb[:],
                                     w2_sb[:, kt * DPAD:(kt + 1) * DPAD],
                                     start=(kt == 0), stop=(kt == 1))
                psc = ppool.tile([128, DPAD], mybir.dt.float16,
                                 name=f"psc_l3_{m}", tag="psc")
                nc.vector.tensor_scalar(psc[:], p_ps[:], dinv_sb[:, m:m + 1],
                                        None, mybir.AluOpType.mult)
                nc.sync.dma_start(ag3_in[m * NPB:(m + 1) * NPB, :], psc[:NPB, :])
            nc.gpsimd.collective_compute(
                "AllGather", mybir.AluOpType.bypass,
                replica_groups=[list(range(NCORES))],
                ins=[ag3_in[:]], outs=[ag3_out[:]])

            for b in range(NBLK):
                acc = gather_block(ag3_out, b, DPAD, "l3")
                y = epool.tile([128, DPAD], mybir.dt.float32, name=f"y3_{b}", tag="y")
                nc.vector.tensor_scalar(y[:], acc[:], dinv_sb[:, b:b + 1],
                                        None, mybir.AluOpType.mult)
                z = epool.tile([128, DOUT], mybir.dt.float32, name=f"z3_{b}", tag="z3")
                nc.vector.tensor_add(z[:], y[:, :DOUT], b2_sb[:])
                # log_softmax over the 40 classes (free dim)
                nmx = epool.tile([128, 1], mybir.dt.float32, name=f"nmx_{b}", tag="r1")
                nc.vector.tensor_reduce(nmx[:], z[:], mybir.AxisListType.X,
                                        mybir.AluOpType.max, negate=True)
                ex = epool.tile([128, DOUT], mybir.dt.float32, name=f"ex_{b}", tag="ex")
                nc.scalar.activation(ex[:], z[:], mybir.ActivationFunctionType.Exp,
                                     bias=nmx[:])
                sm = epool.tile([128, 1], mybir.dt.float32, name=f"sm_{b}", tag="r2")
                nc.vector.tensor_reduce(sm[:], ex[:], mybir.AxisListType.X,
                                        mybir.AluOpType.add)
                ls = epool.tile([128, 1], mybir.dt.float32, name=f"ls_{b}", tag="r3")
                nc.scalar.activation(ls[:], sm[:], mybir.ActivationFunctionType.Ln)
                tot = epool.tile([128, 1], mybir.dt.float32, name=f"tot_{b}", tag="r4")
                nc.vector.tensor_sub(tot[:], nmx[:], ls[:])
                o = epool.tile([128, DOUT], mybir.dt.float32, name=f"o_{b}", tag="o")
                nc.vector.tensor_scalar(o[:], z[:], tot[:], None,
                                        mybir.AluOpType.add)
                nc.sync.dma_start(out_t[b * NPB:(b + 1) * NPB, :], o[:NPB, :])

    nc.compile()
    return nc


def _preprocess(edge_index):
    src = np.asarray(edge_index[0], dtype=np.int64)
    dst = np.asarray(edge_index[1], dtype=np.int64)
    deg = np.bincount(dst, minlength=N).astype(np.float32) + 1.0
    dinv = (1.0 / np.sqrt(deg)).astype(np.float32)

    # rows to gather per node (in-edges + self edge) drive bin cost
    rows_per_node = (deg).astype(np.int64)  # deg already includes +1 (self)
    order = np.argsort(-rows_per_node, kind="stable")
    bin_cost = np.zeros(NBINS, np.int64)
    bin_cnt = np.zeros(NBINS, np.int64)
    node_bin = np.empty(N, np.int64)
    node_pos = np.empty(N, np.int64)
    BIG = 1 << 60
    cost_view = bin_cost.copy()
    for nid in order:
        b = int(np.argmin(cost_view))
        node_bin[nid] = b
        node_pos[nid] = bin_cnt[b]
        bin_cnt[b] += 1
        bin_cost[b] += rows_per_node[nid]
        cost_view[b] = bin_cost[b] if bin_cnt[b] < NPB else BIG
    assert bin_cnt.max() <= NPB and bin_cnt.sum() == N

    perm_row = node_bin * NPB + node_pos  # node -> row in the AG'd feature matrix

    # edge slots per bin: in-edges plus one self edge per node
    e_bin = np.concatenate([node_bin[dst], node_bin])          # [E + N]
    e_srcrow = np.concatenate([perm_row[src], perm_row])       # gather row idx
    e_dpos = np.concatenate([node_pos[dst], node_pos])         # dst col in block
    cnt = np.bincount(e_bin, minlength=NBINS)
    nt = int(np.ceil(cnt.max() / 128))

    # stable order by bin -> slot ids
    eorder = np.argsort(e_bin, kind="stable")
    e_bin = e_bin[eorder]
    e_srcrow = e_srcrow[eorder]
    e_dpos = e_dpos[eorder]
    offs = np.zeros(NBINS + 1, np.int64)
    np.cumsum(cnt, out=offs[1:])
    slot_in_bin = np.arange(len(e_bin)) - offs[e_bin]

    ntt = NBLK * nt
    per_core = []
    for c in range(NCORES):
        mask = (e_bin >= c * NBLK) & (e_bin < (c + 1) * NBLK)
        blk = e_bin[mask] - c * NBLK
        sl = slot_in_bin[mask]
        srow = e_srcrow[mask]
        dpos = e_dpos[mask]
        lin = np.zeros(ntt * 128, np.int16)
        gslot = blk * (nt * 128) + sl
        lin[gslot] = srow.astype(np.int16)
        # wrap layout per block: idx[p, s] = lin_block[s*16 + p%16]
        idx_arr = np.empty((128, ntt * 8), np.int16)
        segs = lin.reshape(NBLK, nt * 8, 16)  # [b, s, 16]
        wrapped = np.transpose(segs, (2, 0, 1)).reshape(16, ntt * 8)
        idx_arr[:] = np.tile(wrapped, (8, 1))
        mt = np.zeros((128, ntt * 128), ml_dtypes.float8_e4m3)
        tg = blk * nt + sl // 128
        mt[sl % 128, tg * 128 + dpos] = 1.0
        per_core.append((idx_arr, mt))

    return dinv, perm_row, nt, per_core


def kernel(x, edge_index, W1, b1, Wh, bh, W2, b2):
    from concourse.bass_utils import run_bass_kernel_spmd

    x = np.asarray(x, np.float32)
    W1 = np.asarray(W1, np.float32)
    b1 = np.asarray(b1, np.float32)
    Wh = np.asarray(Wh, np.float32)
    bh = np.asarray(bh, np.float32)
    W2 = np.asarray(W2, np.float32)
    b2 = np.asarray(b2, np.float32)

    dinv, perm_row, nt, per_core = _preprocess(edge_index)

    if nt not in _CACHE:
        _CACHE[nt] = _build_program(nt)
    nc = _CACHE[nt]

    inv_order = np.argsort(perm_row)  # row -> node

    def wlayout(W, cols):
        wp = np.zeros((D, cols), np.float16)
        wp[:, :W.shape[1]] = W.astype(np.float16)
        return wp.reshape(2, 128, cols).transpose(1, 0, 2).reshape(128, 2 * cols)

    w1h = wlayout(W1, D)
    whh = wlayout(Wh, D)
    w2h = wlayout(W2, DPAD)
    b1t = np.broadcast_to(b1, (128, D)).copy()
    bht = np.broadcast_to(bh, (128, D)).copy()
    b2t = np.broadcast_to(b2, (128, DOUT)).copy()
    ident = np.eye(128, dtype=np.float16)

    in_maps = []
    for c in range(NCORES):
        rows = inv_order[c * NBLK * NPB:(c + 1) * NBLK * NPB]  # node ids by row
        xl = np.zeros((128, NBLK * D), np.float32)
        xc = x[rows].reshape(NBLK, NPB, D)
        xl_v = xl.reshape(128, NBLK, D)
        xl_v[:NPB, :, :] = np.transpose(xc, (1, 0, 2))
        dinvc = np.zeros((128, NBLK), np.float32)
        dinvc[:NPB, :] = dinv[rows].reshape(NBLK, NPB).T
        idx_arr, mt = per_core[c]
        in_maps.append({
            "xl": xl, "w1": w1h, "wh": whh, "w2": w2h,
            "b1t": b1t, "bht": bht, "b2t": b2t,
            "dinvc": dinvc, "idx": idx_arr, "mt": mt, "ident": ident,
        })

    res = run_bass_kernel_spmd(nc, in_maps, core_ids=list(range(NCORES)),
                               trace=TRACE)
    global LAST_RESULT
    LAST_RESULT = res
    full = np.concatenate([res.results[c]["out"] for c in range(NCORES)], axis=0)
    out = np.empty((N, DOUT), np.float32)
    out[inv_order] = full  # row r holds node inv_order[r]
    return out


# revision 9
# speedup vs baseline: 2.5900x; 2.5900x over previous
"""3-layer GCN (message passing) on 8 Trainium2 NeuronCores.

Strategy
--------
Nodes are split into 80 bins (8 cores x 10 blocks, 125 nodes each).
The GCN norm is separable (norm_e = dinv[src]*dinv[dst]), so per layer:

  1. sharded projection  p = h_local @ W  (PE, fp16), rows pre-scaled by
     dinv[node]
  2. AllGather p_scaled -> p_full [10000, D] fp16 in DRAM on every core
  3. aggregation as a DENSE blocked matmul: for each dst block,
     psum[128 dst, D] += M[b,k].T-free @ P[k] over all 79 src k-tiles,
     where M[b,k] is the 0/1 (A+I) adjacency sub-block in fp8 (exact)
     built on the host at kernel-build time and streamed from DRAM,
     and P (the gathered-from tensor) is SBUF-resident fp16.
     This needs zero indirect DMA (the Q7 descriptor-generation path is
     ~8.4 ns/row and was the bottleneck of a gather formulation).
  4. epilogue: relu(psum * dinv[dst] + b)  (DVE/ACT)

Self-loops are ordinary 1-entries in M (the separable scaling gives
exactly dinv^2). Layer 3 ends with a log-softmax over the 40 classes
(free-dim reductions); each core stores its 1250x40 slice.
"""

import numpy as np
import ml_dtypes

N = 10000
E = 320000
D = 256
DOUT = 40
DPAD = 128  # layer-3 projected width padding
NCORES = 8
NBLK = 10       # dst blocks per core
NPB = 125       # nodes per block
NBINS = NCORES * NBLK
NKT = (N + 127) // 128     # 79 src k-tiles
LASTK = N - (NKT - 1) * 128  # 16 rows in the last k-tile

_CACHE = {}

# test.py sets TRACE=True (after installing the NTFF hook shim) to get a
# hardware profile; the graded path leaves this False.
TRACE = False
LAST_RESULT = None


def _build_program():
    import concourse.bacc as bacc
    import concourse.mybir as mybir
    import concourse.tile as tile

    dt = mybir.dt

    nc = bacc.Bacc("TRN2", target_bir_lowering=False, debug=False,
                   num_devices=NCORES)

    # ---- I/O ----
    xl_in = nc.dram_tensor("xl", [128, NBLK * D], dt.float32, kind="ExternalInput")
    w1_in = nc.dram_tensor("w1", [128, 2 * D], dt.float16, kind="ExternalInput")
    wh_in = nc.dram_tensor("wh", [128, 2 * D], dt.float16, kind="ExternalInput")
    w2_in = nc.dram_tensor("w2", [128, 2 * DPAD], dt.float16, kind="ExternalInput")
    b1_in = nc.dram_tensor("b1t", [128, D], dt.float32, kind="ExternalInput")
    bh_in = nc.dram_tensor("bht", [128, D], dt.float32, kind="ExternalInput")
    b2_in = nc.dram_tensor("b2t", [128, DOUT], dt.float32, kind="ExternalInput")
    dinv_in = nc.dram_tensor("dinvc", [128, NBLK], dt.float32, kind="ExternalInput")
    mt_in = nc.dram_tensor("mt", [128, NBLK * NKT * 128], dt.float8e4,
                           kind="ExternalInput")
    id_in = nc.dram_tensor("ident", [128, 128], dt.float16, kind="ExternalInput")
    out_t = nc.dram_tensor("out", [NBLK * NPB, DOUT], dt.float32,
                           kind="ExternalOutput")

    with tile.TileContext(nc) as tc:
        with (
            tc.tile_pool(name="const", bufs=1) as cpool,
            tc.tile_pool(name="hbuf", bufs=1) as hpool,
            tc.tile_pool(name="pres", bufs=2) as prespool,
            tc.tile_pool(name="mslab", bufs=3) as mpool,
            tc.tile_pool(name="tp", bufs=4) as tpool,
            tc.tile_pool(name="psc", bufs=3) as ppool,
            tc.tile_pool(name="epi", bufs=3) as epool,
            tc.tile_pool(name="tp_ps", bufs=4, space="PSUM") as tp_ps,
            tc.tile_pool(name="proj_ps", bufs=2, space="PSUM") as proj_ps,
            tc.tile_pool(name="acc_ps", bufs=2, space="PSUM") as acc_ps,
            tc.tile_pool(name="dram", bufs=1, space="DRAM") as dram,
        ):
            # ---- load constants ----
            ident = cpool.tile([128, 128], dt.float16)
            w1_sb = cpool.tile([128, 2 * D], dt.float16)
            wh_sb = cpool.tile([128, 2 * D], dt.float16)
            w2_sb = cpool.tile([128, 2 * DPAD], dt.float16)
            b1_sb = cpool.tile([128, D], dt.float32)
            bh_sb = cpool.tile([128, D], dt.float32)
            b2_sb = cpool.tile([128, DOUT], dt.float32)
            dinv_sb = cpool.tile([128, NBLK], dt.float32)
            nc.sync.dma_start(ident[:], id_in[:])
            nc.sync.dma_start(w1_sb[:], w1_in[:])
            nc.sync.dma_start(wh_sb[:], wh_in[:])
            nc.sync.dma_start(w2_sb[:], w2_in[:])
            nc.sync.dma_start(b1_sb[:], b1_in[:])
            nc.sync.dma_start(bh_sb[:], bh_in[:])
            nc.sync.dma_start(b2_sb[:], b2_in[:])
            nc.sync.dma_start(dinv_sb[:], dinv_in[:])

            # ---- x load + fp16 convert ----
            x32 = hpool.tile([128, NBLK * D], dt.float32)
            nc.sync.dma_start(x32[:], xl_in[:])
            xh = hpool.tile([128, NBLK * D], dt.float16)
            nc.vector.tensor_copy(xh[:], x32[:])

            def project(src_sb, w_sb, dcols, ag_in, tag):
                """p_scaled[m] = (src[:, m] @ W) * dinv -> ag_in rows."""
                for m in range(NBLK):
                    p_ps = proj_ps.tile([128, dcols], mybir.dt.float32,
                                        name=f"p_ps_{tag}_{m}", tag="p_ps")
                    for kt in range(2):
                        t_ps = tp_ps.tile([128, 128], mybir.dt.float16,
                                          name=f"t_ps_{tag}_{m}_{kt}", tag="t_ps")
                        nc.tensor.transpose(
                            t_ps[:], src_sb[:, m * D + kt * 128: m * D + (kt + 1) * 128],
                            ident[:])
                        t_sb = tpool.tile([128, 128], mybir.dt.float16,
                                          name=f"t_sb_{tag}_{m}_{kt}", tag="t_sb")
                        nc.vector.tensor_copy(t_sb[:], t_ps[:])
                        nc.tensor.matmul(p_ps[:], t_sb[:],
                                         w_sb[:, kt * dcols:(kt + 1) * dcols],
                                         start=(kt == 0), stop=(kt == 1))
                    psc = ppool.tile([128, dcols], mybir.dt.float16,
                                     name=f"psc_{tag}_{m}", tag="psc")
                    nc.vector.tensor_scalar(psc[:], p_ps[:], dinv_sb[:, m:m + 1],
                                            None, mybir.AluOpType.mult)
                    nc.sync.dma_start(ag_in[m * NPB:(m + 1) * NPB, :], psc[:NPB, :])

            def load_p(ag_out, dcols, tag):
                """AG output [N, dcols] fp16 -> SBUF-resident [128, NKT*dcols]
                (k-tile-major: column block k holds rows k*128..k*128+127)."""
                p_sb = prespool.tile([128, NKT * dcols], mybir.dt.float16,
                                     name=f"p_{tag}", tag="pres")
                nchunk = 6
                per = (NKT - 1 + nchunk - 1) // nchunk
                for ci in range(nchunk):
                    k0 = ci * per
                    k1 = min((ci + 1) * per, NKT - 1)
                    if k0 >= k1:
                        break
                    nc.sync.dma_start(
                        p_sb[:, k0 * dcols: k1 * dcols].rearrange(
                            "p (k d) -> p k d", d=dcols),
                        ag_out[k0 * 128: k1 * 128, :].rearrange(
                            "(k p) d -> p k d", p=128))
                nc.sync.dma_start(
                    p_sb[:LASTK, (NKT - 1) * dcols: NKT * dcols],
                    ag_out[(NKT - 1) * 128:, :])
                return p_sb

            def agg_block(p_sb, b, dcols, tag):
                """psum[128 dst, dcols] = sum_k M[b,k] @ P[k]."""
                mslab = mpool.tile([128, NKT * 128], mybir.dt.float8e4,
                                   name=f"m_{tag}_{b}", tag="mslab")
                nc.sync.dma_start(
                    mslab[:], mt_in[:, b * NKT * 128:(b + 1) * NKT * 128])
                acc = acc_ps.tile([128, dcols], mybir.dt.float32,
                                  name=f"acc_{tag}_{b}", tag="acc")
                for k in range(NKT):
                    ksz = 128 if k < NKT - 1 else LASTK
                    nc.tensor.matmul(
                        acc[:], mslab[:ksz, k * 128:(k + 1) * 128],
                        p_sb[:ksz, k * dcols:(k + 1) * dcols],
                        start=(k == 0), stop=(k == NKT - 1))
                return acc

            # ================= layer 1 =================
            ag1_in = dram.tile([NBLK * NPB, D], dt.float16)
            ag1_out = dram.tile([N, D], dt.float16)
            project(xh, w1_sb, D, ag1_in, "l1")
            nc.gpsimd.collective_compute(
                "AllGather", mybir.AluOpType.bypass,
                replica_groups=[list(range(NCORES))],
                ins=[ag1_in[:]], outs=[ag1_out[:]])

            h1 = hpool.tile([128, NBLK * D], dt.float16)
            p1_sb = load_p(ag1_out, D, "l1")
            for b in range(NBLK):
                acc = agg_block(p1_sb, b, D, "l1")
                y = epool.tile([128, D], mybir.dt.float32, name=f"y1_{b}", tag="y")
                nc.vector.tensor_scalar(y[:], acc[:], dinv_sb[:, b:b + 1],
                                        None, mybir.AluOpType.mult)
                z = epool.tile([128, D], mybir.dt.float32, name=f"z1_{b}", tag="z")
                nc.vector.tensor_add(z[:], y[:], b1_sb[:])
                nc.scalar.activation(h1[:, b * D:(b + 1) * D], z[:],
                                     mybir.ActivationFunctionType.Relu)

            # ================= layer 2 =================
            ag2_in = dram.tile([NBLK * NPB, D], dt.float16)
            ag2_out = dram.tile([N, D], dt.float16)
            project(h1, wh_sb, D, ag2_in, "l2")
            nc.gpsimd.collective_compute(
                "AllGather", mybir.AluOpType.bypass,
                replica_groups=[list(range(NCORES))],
                ins=[ag2_in[:]], outs=[ag2_out[:]])

            h2 = hpool.tile([128, NBLK * D], dt.float16)
            p2_sb = load_p(ag2_out, D, "l2")
            for b in range(NBLK):
                acc = agg_block(p2_sb, b, D, "l2")
                y = epool.tile([128, D], mybir.dt.float32, name=f"y2_{b}", tag="y")
                nc.vector.tensor_scalar(y[:], acc[:], dinv_sb[:, b:b + 1],
                                        None, mybir.AluOpType.mult)
                z = epool.tile([128, D], mybir.dt.float32, name=f"z2_{b}", tag="z")
                nc.vector.tensor_add(z[:], y[:], bh_sb[:])
                nc.scalar.activation(h2[:, b * D:(b + 1) * D], z[:],
                                     mybir.ActivationFunctionType.Relu)

            # ================= layer 3 =================
            ag3_in = dram.tile([NBLK * NPB, DPAD], dt.float16)
            ag3_out = dram.tile([N, DPAD], dt.float16)
            for m in range(NBLK):
                p_ps = proj_ps.tile([128, DPAD], mybir.dt.float32,
                                    name=f"p_ps_l3_{m}", tag="p_ps")
                for kt in range(2):
                    t_ps = tp_ps.tile([128, 128], mybir.dt.float16,
                                      name=f"t_ps_l3_{m}_{kt}", tag="t_ps")
                    nc.tensor.transpose(
                        t_ps[:], h2[:, m * D + kt * 128: m * D + (kt + 1) * 128],
                        ident[:])
                    t_sb = tpool.tile([128, 128], mybir.dt.float16,
                                      name=f"t_sb_l3_{m}_{kt}", tag="t_sb")
                    nc.vector.tensor_copy(t_sb[:], t_ps[:])
                    nc.tensor.matmul(p_ps[:], t_sb[:],
                                     w2_sb[:, kt * DPAD:(kt + 1) * DPAD],
                                     start=(kt == 0), stop=(kt == 1))
                psc = ppool.tile([128, DPAD], mybir.dt.float16,
                                 name=f"psc_l3_{m}", tag="psc")
                nc.vector.tensor_scalar(psc[:], p_ps[:], dinv_sb[:, m:m + 1],
                                        None, mybir.AluOpType.mult)
                nc.sync.dma_start(ag3_in[m * NPB:(m + 1) * NPB, :], psc[:NPB, :])
            nc.gpsimd.collective_compute(
                "AllGather", mybir.AluOpType.bypass,
                replica_groups=[list(range(NCORES))],
                ins=[ag3_in[:]], outs=[ag3_out[:]])

            p3_sb = load_p(ag3_out, DPAD, "l3")
            for b in range(NBLK):
                acc = agg_block(p3_sb, b, DPAD, "l3")
                y = epool.tile([128, DPAD], mybir.dt.float32, name=f"y3_{b}", tag="y")
                nc.vector.tensor_scalar(y[:], acc[:], dinv_sb[:, b:b + 1],
                                        None, mybir.AluOpType.mult)
                z = epool.tile([128, DOUT], mybir.dt.float32, name=f"z3_{b}", tag="z3")
                nc.vector.tensor_add(z[:], y[:, :DOUT], b2_sb[:])
                # log_softmax over the 40 classes (free dim)
                nmx = epool.tile([128, 1], mybir.dt.float32, name=f"nmx_{b}", tag="r1")
                nc.vector.tensor_reduce(nmx[:], z[:], mybir.AxisListType.X,
                                        mybir.AluOpType.max, negate=True)
                ex = epool.tile([128, DOUT], mybir.dt.float32, name=f"ex_{b}", tag="ex")
                nc.scalar.activation(ex[:], z[:], mybir.ActivationFunctionType.Exp,
                                     bias=nmx[:])
                sm = epool.tile([128, 1], mybir.dt.float32, name=f"sm_{b}", tag="r2")
                nc.vector.tensor_reduce(sm[:], ex[:], mybir.AxisListType.X,
                                        mybir.AluOpType.add)
                ls = epool.tile([128, 1], mybir.dt.float32, name=f"ls_{b}", tag="r3")
                nc.scalar.activation(ls[:], sm[:], mybir.ActivationFunctionType.Ln)
                tot = epool.tile([128, 1], mybir.dt.float32, name=f"tot_{b}", tag="r4")
                nc.vector.tensor_sub(tot[:], nmx[:], ls[:])
                o = epool.tile([128, DOUT], mybir.dt.float32, name=f"o_{b}", tag="o")
                nc.vector.tensor_scalar(o[:], z[:], tot[:], None,
                                        mybir.AluOpType.add)
                nc.sync.dma_start(out_t[b * NPB:(b + 1) * NPB, :], o[:NPB, :])

    nc.compile()
    return nc


def _preprocess(edge_index):
    src = np.asarray(edge_index[0], dtype=np.int64)
    dst = np.asarray(edge_index[1], dtype=np.int64)
    deg = np.bincount(dst, minlength=N).astype(np.float32) + 1.0
    dinv = (1.0 / np.sqrt(deg)).astype(np.float32)

    # simple contiguous binning (dense aggregation cost is shape-uniform)
    node_bin = np.arange(N) // NPB
    node_pos = np.arange(N) % NPB
    perm_row = node_bin * NPB + node_pos  # == identity here

    # dense 0/1 (A+I) blocks, fp8: per core [128 src_local, NBLK*NKT*128]
    srow = perm_row[src]
    dbin = node_bin[dst]
    dpos = node_pos[dst]
    # self loops
    srow_all = np.concatenate([srow, perm_row])
    dbin_all = np.concatenate([dbin, node_bin])
    dpos_all = np.concatenate([dpos, node_pos])

    per_core = []
    for c in range(NCORES):
        mask = (dbin_all >= c * NBLK) & (dbin_all < (c + 1) * NBLK)
        sr = srow_all[mask]
        b = dbin_all[mask] - c * NBLK
        dp = dpos_all[mask]
        m = np.zeros((128, NBLK * NKT * 128), np.uint8)
        cols = (b * NKT + sr // 128) * 128 + dp
        np.add.at(m, (sr % 128, cols), 1)
        assert m.max() <= 8, "fp8 exact-int limit exceeded"
        per_core.append(m.astype(ml_dtypes.float8_e4m3))

    return dinv, perm_row, per_core


def kernel(x, edge_index, W1, b1, Wh, bh, W2, b2):
    from concourse.bass_utils import run_bass_kernel_spmd

    x = np.asarray(x, np.float32)
    W1 = np.asarray(W1, np.float32)
    b1 = np.asarray(b1, np.float32)
    Wh = np.asarray(Wh, np.float32)
    bh = np.asarray(bh, np.float32)
    W2 = np.asarray(W2, np.float32)
    b2 = np.asarray(b2, np.float32)

    dinv, perm_row, per_core = _preprocess(edge_index)

    if "prog" not in _CACHE:
        _CACHE["prog"] = _build_program()
    nc = _CACHE["prog"]

    inv_order = np.argsort(perm_row)  # row -> node

    def wlayout(W, cols):
        wp = np.zeros((D, cols), np.float16)
        wp[:, :W.shape[1]] = W.astype(np.float16)
        return wp.reshape(2, 128, cols).transpose(1, 0, 2).reshape(128, 2 * cols)

    w1h = wlayout(W1, D)
    whh = wlayout(Wh, D)
    w2h = wlayout(W2, DPAD)
    b1t = np.broadcast_to(b1, (128, D)).copy()
    bht = np.broadcast_to(bh, (128, D)).copy()
    b2t = np.broadcast_to(b2, (128, DOUT)).copy()
    ident = np.eye(128, dtype=np.float16)

    in_maps = []
    for c in range(NCORES):
        rows = inv_order[c * NBLK * NPB:(c + 1) * NBLK * NPB]  # node ids by row
        xl = np.zeros((128, NBLK * D), np.float32)
        xl_v = xl.reshape(128, NBLK, D)
        xl_v[:NPB, :, :] = np.transpose(x[rows].reshape(NBLK, NPB, D), (1, 0, 2))
        dinvc = np.zeros((128, NBLK), np.float32)
        dinvc[:NPB, :] = dinv[rows].reshape(NBLK, NPB).T
        in_maps.append({
            "xl": xl, "w1": w1h, "wh": whh, "w2": w2h,
            "b1t": b1t, "bht": bht, "b2t": b2t,
            "dinvc": dinvc, "mt": per_core[c], "ident": ident,
        })

    res = run_bass_kernel_spmd(nc, in_maps, core_ids=list(range(NCORES)),
                               trace=TRACE)
    global LAST_RESULT
    LAST_RESULT = res
    full = np.concatenate([res.results[c]["out"] for c in range(NCORES)], axis=0)
    out = np.empty((N, DOUT), np.float32)
    out[inv_order] = full  # row r holds node inv_order[r]
    return out


# revision 10
# speedup vs baseline: 2.5971x; 1.0027x over previous
"""3-layer GCN (message passing) on 8 Trainium2 NeuronCores.

Strategy
--------
Nodes are split into 80 bins (8 cores x 10 blocks, 125 nodes each).
The GCN norm is separable (norm_e = dinv[src]*dinv[dst]), so per layer:

  1. sharded projection  p = h_local @ W  (PE, fp16), rows pre-scaled by
     dinv[node]
  2. AllGather p_scaled -> p_full [10000, D] fp16 in DRAM on every core
  3. aggregation as a DENSE blocked matmul: for each dst block,
     psum[128 dst, D] += M[b,k].T-free @ P[k] over all 79 src k-tiles,
     where M[b,k] is the 0/1 (A+I) adjacency sub-block in fp8 (exact)
     built on the host at kernel-build time and streamed from DRAM,
     and P (the gathered-from tensor) is SBUF-resident fp16.
     This needs zero indirect DMA (the Q7 descriptor-generation path is
     ~8.4 ns/row and was the bottleneck of a gather formulation).
  4. epilogue: relu(psum * dinv[dst] + b)  (DVE/ACT)

Self-loops are ordinary 1-entries in M (the separable scaling gives
exactly dinv^2). Layer 3 ends with a log-softmax over the 40 classes
(free-dim reductions); each core stores its 1250x40 slice.
"""

import numpy as np
import ml_dtypes

N = 10000
E = 320000
D = 256
DOUT = 40
DPAD = 128  # layer-3 projected width padding
NCORES = 8
NBLK = 10       # dst blocks per core
NPB = 125       # nodes per block
NBINS = NCORES * NBLK
NKT = (N + 127) // 128     # 79 src k-tiles
LASTK = N - (NKT - 1) * 128  # 16 rows in the last k-tile

_CACHE = {}

# test.py sets TRACE=True (after installing the NTFF hook shim) to get a
# hardware profile; the graded path leaves this False.
TRACE = False
LAST_RESULT = None


def _build_program():
    import concourse.bacc as bacc
    import concourse.mybir as mybir
    import concourse.tile as tile

    dt = mybir.dt

    nc = bacc.Bacc("TRN2", target_bir_lowering=False, debug=False,
                   num_devices=NCORES)

    # ---- I/O ----
    xl_in = nc.dram_tensor("xl", [128, NBLK * D], dt.float32, kind="ExternalInput")
    w1_in = nc.dram_tensor("w1", [128, 2 * D], dt.float16, kind="ExternalInput")
    wh_in = nc.dram_tensor("wh", [128, 2 * D], dt.float16, kind="ExternalInput")
    w2_in = nc.dram_tensor("w2", [128, 2 * DPAD], dt.float16, kind="ExternalInput")
    b1_in = nc.dram_tensor("b1t", [128, D], dt.float32, kind="ExternalInput")
    bh_in = nc.dram_tensor("bht", [128, D], dt.float32, kind="ExternalInput")
    b2_in = nc.dram_tensor("b2t", [128, DOUT], dt.float32, kind="ExternalInput")
    dinv_in = nc.dram_tensor("dinvc", [128, NBLK], dt.float32, kind="ExternalInput")
    mt_in = nc.dram_tensor("mt", [128, NBLK * NKT * 128], dt.float8e4,
                           kind="ExternalInput")
    id_in = nc.dram_tensor("ident", [128, 128], dt.float16, kind="ExternalInput")
    out_t = nc.dram_tensor("out", [NBLK * NPB, DOUT], dt.float32,
                           kind="ExternalOutput")

    with tile.TileContext(nc) as tc:
        with (
            tc.tile_pool(name="const", bufs=1) as cpool,
            tc.tile_pool(name="hbuf", bufs=1) as hpool,
            tc.tile_pool(name="pres", bufs=2) as prespool,
            tc.tile_pool(name="mslab", bufs=3) as mpool,
            tc.tile_pool(name="tp", bufs=4) as tpool,
            tc.tile_pool(name="psc", bufs=3) as ppool,
            tc.tile_pool(name="epi", bufs=3) as epool,
            tc.tile_pool(name="tp_ps", bufs=4, space="PSUM") as tp_ps,
            tc.tile_pool(name="proj_ps", bufs=2, space="PSUM") as proj_ps,
            tc.tile_pool(name="acc_ps", bufs=2, space="PSUM") as acc_ps,
            tc.tile_pool(name="dram", bufs=1, space="DRAM") as dram,
        ):
            # ---- load constants ----
            ident = cpool.tile([128, 128], dt.float16)
            w1_sb = cpool.tile([128, 2 * D], dt.float16)
            wh_sb = cpool.tile([128, 2 * D], dt.float16)
            w2_sb = cpool.tile([128, 2 * DPAD], dt.float16)
            b1_sb = cpool.tile([128, D], dt.float32)
            bh_sb = cpool.tile([128, D], dt.float32)
            b2_sb = cpool.tile([128, DOUT], dt.float32)
            dinv_sb = cpool.tile([128, NBLK], dt.float32)
            nc.sync.dma_start(ident[:], id_in[:])
            nc.sync.dma_start(w1_sb[:], w1_in[:])
            nc.sync.dma_start(wh_sb[:], wh_in[:])
            nc.sync.dma_start(w2_sb[:], w2_in[:])
            nc.sync.dma_start(b1_sb[:], b1_in[:])
            nc.sync.dma_start(bh_sb[:], bh_in[:])
            nc.sync.dma_start(b2_sb[:], b2_in[:])
            nc.sync.dma_start(dinv_sb[:], dinv_in[:])

            # ---- x load + fp16 convert ----
            x32 = hpool.tile([128, NBLK * D], dt.float32)
            nc.sync.dma_start(x32[:], xl_in[:])
            xh = hpool.tile([128, NBLK * D], dt.float16)
            nc.vector.tensor_copy(xh[:], x32[:])

            # Dummy rendezvous: absorb cross-core NEFF-start skew and the
            # first-collective barrier while the projection pipeline runs.
            rdv_in = dram.tile([16, 4], dt.float32)
            rdv_out = dram.tile([128, 4], dt.float32)
            rdv_sb = cpool.tile([16, 4], dt.float32)
            nc.vector.tensor_copy(rdv_sb[:], x32[:16, :4])
            nc.sync.dma_start(rdv_in[:], rdv_sb[:])
            nc.gpsimd.collective_compute(
                "AllGather", mybir.AluOpType.bypass,
                replica_groups=[list(range(NCORES))],
                ins=[rdv_in[:]], outs=[rdv_out[:]])

            def project(src_sb, w_sb, dcols, ag_in, tag):
                """p_scaled[m] = (src[:, m] @ W) * dinv -> ag_in rows."""
                for m in range(NBLK):
                    p_ps = proj_ps.tile([128, dcols], mybir.dt.float32,
                                        name=f"p_ps_{tag}_{m}", tag="p_ps")
                    for kt in range(2):
                        t_ps = tp_ps.tile([128, 128], mybir.dt.float16,
                                          name=f"t_ps_{tag}_{m}_{kt}", tag="t_ps")
                        nc.tensor.transpose(
                            t_ps[:], src_sb[:, m * D + kt * 128: m * D + (kt + 1) * 128],
                            ident[:])
                        t_sb = tpool.tile([128, 128], mybir.dt.float16,
                                          name=f"t_sb_{tag}_{m}_{kt}", tag="t_sb")
                        nc.vector.tensor_copy(t_sb[:], t_ps[:])
                        nc.tensor.matmul(p_ps[:], t_sb[:],
                                         w_sb[:, kt * dcols:(kt + 1) * dcols],
                                         start=(kt == 0), stop=(kt == 1))
                    psc = ppool.tile([128, dcols], mybir.dt.float16,
                                     name=f"psc_{tag}_{m}", tag="psc")
                    nc.vector.tensor_scalar(psc[:], p_ps[:], dinv_sb[:, m:m + 1],
                                            None, mybir.AluOpType.mult)
                    nc.sync.dma_start(ag_in[m * NPB:(m + 1) * NPB, :], psc[:NPB, :])

            def load_p(ag_out, dcols, tag):
                """AG output [N, dcols] fp16 -> SBUF-resident [128, NKT*dcols]
                (k-tile-major: column block k holds rows k*128..k*128+127)."""
                p_sb = prespool.tile([128, NKT * dcols], mybir.dt.float16,
                                     name=f"p_{tag}", tag="pres")
                nchunk = 6
                per = (NKT - 1 + nchunk - 1) // nchunk
                for ci in range(nchunk):
                    k0 = ci * per
                    k1 = min((ci + 1) * per, NKT - 1)
                    if k0 >= k1:
                        break
                    nc.sync.dma_start(
                        p_sb[:, k0 * dcols: k1 * dcols].rearrange(
                            "p (k d) -> p k d", d=dcols),
                        ag_out[k0 * 128: k1 * 128, :].rearrange(
                            "(k p) d -> p k d", p=128))
                nc.sync.dma_start(
                    p_sb[:LASTK, (NKT - 1) * dcols: NKT * dcols],
                    ag_out[(NKT - 1) * 128:, :])
                return p_sb

            def agg_block(p_sb, b, dcols, tag):
                """psum[128 dst, dcols] = sum_k M[b,k] @ P[k]."""
                mslab = mpool.tile([128, NKT * 128], mybir.dt.float8e4,
                                   name=f"m_{tag}_{b}", tag="mslab")
                nc.sync.dma_start(
                    mslab[:], mt_in[:, b * NKT * 128:(b + 1) * NKT * 128])
                acc = acc_ps.tile([128, dcols], mybir.dt.float32,
                                  name=f"acc_{tag}_{b}", tag="acc")
                for k in range(NKT):
                    ksz = 128 if k < NKT - 1 else LASTK
                    nc.tensor.matmul(
                        acc[:], mslab[:ksz, k * 128:(k + 1) * 128],
                        p_sb[:ksz, k * dcols:(k + 1) * dcols],
                        start=(k == 0), stop=(k == NKT - 1))
                return acc

            # ================= layer 1 =================
            ag1_in = dram.tile([NBLK * NPB, D], dt.float16)
            ag1_out = dram.tile([N, D], dt.float16)
            project(xh, w1_sb, D, ag1_in, "l1")
            nc.gpsimd.collective_compute(
                "AllGather", mybir.AluOpType.bypass,
                replica_groups=[list(range(NCORES))],
                ins=[ag1_in[:]], outs=[ag1_out[:]])

            h1 = hpool.tile([128, NBLK * D], dt.float16)
            p1_sb = load_p(ag1_out, D, "l1")
            for b in range(NBLK):
                acc = agg_block(p1_sb, b, D, "l1")
                y = epool.tile([128, D], mybir.dt.float32, name=f"y1_{b}", tag="y")
                nc.vector.tensor_scalar(y[:], acc[:], dinv_sb[:, b:b + 1],
                                        None, mybir.AluOpType.mult)
                z = epool.tile([128, D], mybir.dt.float32, name=f"z1_{b}", tag="z")
                nc.vector.tensor_add(z[:], y[:], b1_sb[:])
                nc.scalar.activation(h1[:, b * D:(b + 1) * D], z[:],
                                     mybir.ActivationFunctionType.Relu)

            # ================= layer 2 =================
            ag2_in = dram.tile([NBLK * NPB, D], dt.float16)
            ag2_out = dram.tile([N, D], dt.float16)
            project(h1, wh_sb, D, ag2_in, "l2")
            nc.gpsimd.collective_compute(
                "AllGather", mybir.AluOpType.bypass,
                replica_groups=[list(range(NCORES))],
                ins=[ag2_in[:]], outs=[ag2_out[:]])

            h2 = hpool.tile([128, NBLK * D], dt.float16)
            p2_sb = load_p(ag2_out, D, "l2")
            for b in range(NBLK):
                acc = agg_block(p2_sb, b, D, "l2")
                y = epool.tile([128, D], mybir.dt.float32, name=f"y2_{b}", tag="y")
                nc.vector.tensor_scalar(y[:], acc[:], dinv_sb[:, b:b + 1],
                                        None, mybir.AluOpType.mult)
                z = epool.tile([128, D], mybir.dt.float32, name=f"z2_{b}", tag="z")
                nc.vector.tensor_add(z[:], y[:], bh_sb[:])
                nc.scalar.activation(h2[:, b * D:(b + 1) * D], z[:],
                                     mybir.ActivationFunctionType.Relu)

            # ================= layer 3 =================
            ag3_in = dram.tile([NBLK * NPB, DPAD], dt.float16)
            ag3_out = dram.tile([N, DPAD], dt.float16)
            for m in range(NBLK):
                p_ps = proj_ps.tile([128, DPAD], mybir.dt.float32,
                                    name=f"p_ps_l3_{m}", tag="p_ps")
                for kt in range(2):
                    t_ps = tp_ps.tile([128, 128], mybir.dt.float16,
                                      name=f"t_ps_l3_{m}_{kt}", tag="t_ps")
                    nc.tensor.transpose(
                        t_ps[:], h2[:, m * D + kt * 128: m * D + (kt + 1) * 128],
                        ident[:])
                    t_sb = tpool.tile([128, 128], mybir.dt.float16,
                                      name=f"t_sb_l3_{m}_{kt}", tag="t_sb")
                    nc.vector.tensor_copy(t_sb[:], t_ps[:])
                    nc.tensor.matmul(p_ps[:], t_sb[:],
                                     w2_sb[:, kt * DPAD:(kt + 1) * DPAD],
                                     start=(kt == 0), stop=(kt == 1))
                psc = ppool.tile([128, DPAD], mybir.dt.float16,
                                 name=f"psc_l3_{m}", tag="psc")
                nc.vector.tensor_scalar(psc[:], p_ps[:], dinv_sb[:, m:m + 1],
                                        None, mybir.AluOpType.mult)
                nc.sync.dma_start(ag3_in[m * NPB:(m + 1) * NPB, :], psc[:NPB, :])
            nc.gpsimd.collective_compute(
                "AllGather", mybir.AluOpType.bypass,
                replica_groups=[list(range(NCORES))],
                ins=[ag3_in[:]], outs=[ag3_out[:]])

            p3_sb = load_p(ag3_out, DPAD, "l3")
            for b in range(NBLK):
                acc = agg_block(p3_sb, b, DPAD, "l3")
                y = epool.tile([128, DPAD], mybir.dt.float32, name=f"y3_{b}", tag="y")
                nc.vector.tensor_scalar(y[:], acc[:], dinv_sb[:, b:b + 1],
                                        None, mybir.AluOpType.mult)
                z = epool.tile([128, DOUT], mybir.dt.float32, name=f"z3_{b}", tag="z3")
                nc.vector.tensor_add(z[:], y[:, :DOUT], b2_sb[:])
                # log_softmax over the 40 classes (free dim)
                nmx = epool.tile([128, 1], mybir.dt.float32, name=f"nmx_{b}", tag="r1")
                nc.vector.tensor_reduce(nmx[:], z[:], mybir.AxisListType.X,
                                        mybir.AluOpType.max, negate=True)
                ex = epool.tile([128, DOUT], mybir.dt.float32, name=f"ex_{b}", tag="ex")
                nc.scalar.activation(ex[:], z[:], mybir.ActivationFunctionType.Exp,
                                     bias=nmx[:])
                sm = epool.tile([128, 1], mybir.dt.float32, name=f"sm_{b}", tag="r2")
                nc.vector.tensor_reduce(sm[:], ex[:], mybir.AxisListType.X,
                                        mybir.AluOpType.add)
                ls = epool.tile([128, 1], mybir.dt.float32, name=f"ls_{b}", tag="r3")
                nc.scalar.activation(ls[:], sm[:], mybir.ActivationFunctionType.Ln)
                tot = epool.tile([128, 1], mybir.dt.float32, name=f"tot_{b}", tag="r4")
                nc.vector.tensor_sub(tot[:], nmx[:], ls[:])
                o = epool.tile([128, DOUT], mybir.dt.float32, name=f"o_{b}", tag="o")
                nc.vector.tensor_scalar(o[:], z[:], tot[:], None,
                                        mybir.AluOpType.add)
                nc.sync.dma_start(out_t[b * NPB:(b + 1) * NPB, :], o[:NPB, :])

    nc.compile()
    return nc


def _preprocess(edge_index):
    src = np.asarray(edge_index[0], dtype=np.int64)
    dst = np.asarray(edge_index[1], dtype=np.int64)
    deg = np.bincount(dst, minlength=N).astype(np.float32) + 1.0
    dinv = (1.0 / np.sqrt(deg)).astype(np.float32)

    # simple contiguous binning (dense aggregation cost is shape-uniform)
    node_bin = np.arange(N) // NPB
    node_pos = np.arange(N) % NPB
    perm_row = node_bin * NPB + node_pos  # == identity here

    # dense 0/1 (A+I) blocks, fp8: per core [128 src_local, NBLK*NKT*128]
    srow = perm_row[src]
    dbin = node_bin[dst]
    dpos = node_pos[dst]
    # self loops
    srow_all = np.concatenate([srow, perm_row])
    dbin_all = np.concatenate([dbin, node_bin])
    dpos_all = np.concatenate([dpos, node_pos])

    per_core = []
    for c in range(NCORES):
        mask = (dbin_all >= c * NBLK) & (dbin_all < (c + 1) * NBLK)
        sr = srow_all[mask]
        b = dbin_all[mask] - c * NBLK
        dp = dpos_all[mask]
        m = np.zeros((128, NBLK * NKT * 128), np.uint8)
        cols = (b * NKT + sr // 128) * 128 + dp
        np.add.at(m, (sr % 128, cols), 1)
        assert m.max() <= 8, "fp8 exact-int limit exceeded"
        per_core.append(m.astype(ml_dtypes.float8_e4m3))

    return dinv, perm_row, per_core


def kernel(x, edge_index, W1, b1, Wh, bh, W2, b2):
    from concourse.bass_utils import run_bass_kernel_spmd

    x = np.asarray(x, np.float32)
    W1 = np.asarray(W1, np.float32)
    b1 = np.asarray(b1, np.float32)
    Wh = np.asarray(Wh, np.float32)
    bh = np.asarray(bh, np.float32)
    W2 = np.asarray(W2, np.float32)
    b2 = np.asarray(b2, np.float32)

    dinv, perm_row, per_core = _preprocess(edge_index)

    if "prog" not in _CACHE:
        _CACHE["prog"] = _build_program()
    nc = _CACHE["prog"]

    inv_order = np.argsort(perm_row)  # row -> node

    def wlayout(W, cols):
        wp = np.zeros((D, cols), np.float16)
        wp[:, :W.shape[1]] = W.astype(np.float16)
        return wp.reshape(2, 128, cols).transpose(1, 0, 2).reshape(128, 2 * cols)

    w1h = wlayout(W1, D)
    whh = wlayout(Wh, D)
    w2h = wlayout(W2, DPAD)
    b1t = np.broadcast_to(b1, (128, D)).copy()
    bht = np.broadcast_to(bh, (128, D)).copy()
    b2t = np.broadcast_to(b2, (128, DOUT)).copy()
    ident = np.eye(128, dtype=np.float16)

    in_maps = []
    for c in range(NCORES):
        rows = inv_order[c * NBLK * NPB:(c + 1) * NBLK * NPB]  # node ids by row
        xl = np.zeros((128, NBLK * D), np.float32)
        xl_v = xl.reshape(128, NBLK, D)
        xl_v[:NPB, :, :] = np.transpose(x[rows].reshape(NBLK, NPB, D), (1, 0, 2))
        dinvc = np.zeros((128, NBLK), np.float32)
        dinvc[:NPB, :] = dinv[rows].reshape(NBLK, NPB).T
        in_maps.append({
            "xl": xl, "w1": w1h, "wh": whh, "w2": w2h,
            "b1t": b1t, "bht": bht, "b2t": b2t,
            "dinvc": dinvc, "mt": per_core[c], "ident": ident,
        })

    res = run_bass_kernel_spmd(nc, in_maps, core_ids=list(range(NCORES)),
                               trace=TRACE)
    global LAST_RESULT
    LAST_RESULT = res
    full = np.concatenate([res.results[c]["out"] for c in range(NCORES)], axis=0)
    out = np.empty((N, DOUT), np.float32)
    out[inv_order] = full  # row r holds node inv_order[r]
    return out
